# revision 41
# baseline (speedup 1.0000x reference)
"""BiasAttention TRN2 kernel — q-sharded, fp8 z, baseline loop structure.

Known-good probe variant (the 114us run): fp8-e3m4 z, gk=64 DMA groups,
HAM warmup, original S-prologue + in-loop bias/add/exp/transpose/AV.
"""

import sys

if "/opt/trn_rl_repo" not in sys.path:
    sys.path.insert(0, "/opt/trn_rl_repo")

import ml_dtypes
import numpy as np

import concourse.bass as bass
import concourse.mybir as mybir
from concourse import bacc
from concourse.bass_utils import run_bass_kernel_spmd
from concourse.masks import make_identity
from concourse.tile import TileContext

P = 128
H = 8
D = 32
CQ = 256
CKV = 256
BD = 128
NQ = 1024
NCORES = 8
NQC = NQ // NCORES
SCALE = D ** (-0.5)

GK = 64
FP = mybir.dt.float32
BF = mybir.dt.bfloat16
F8 = mybir.dt.float8e3
NP_BF = ml_dtypes.bfloat16
NP_F8 = ml_dtypes.float8_e3m4

Z_SCALE = 2.0
WB_SCALE = 32.0
BIAS_SCALE = Z_SCALE * WB_SCALE


def build_program(nk=1024, gk=GK):
    kc_n = nk // P
    ng = nk // gk
    add = mybir.AluOpType.add
    mult = mybir.AluOpType.mult

    nc = bacc.Bacc("TRN2", target_bir_lowering=False, debug=False,
                   num_devices=NCORES)

    zT = nc.dram_tensor("zT", [ng, BD, gk, NQC], F8, kind="ExternalInput")
    xqT = nc.dram_tensor("xqT", [CQ, NQC], BF, kind="ExternalInput")
    xkvT = nc.dram_tensor("xkvT", [CKV, nk], BF, kind="ExternalInput")
    Wq = nc.dram_tensor("Wq", [CQ, H * D], BF, kind="ExternalInput")
    bq = nc.dram_tensor("bq", [H * D], FP, kind="ExternalInput")
    Wkv = nc.dram_tensor("Wkv", [CKV, 2 * H * D], BF, kind="ExternalInput")
    bkv = nc.dram_tensor("bkv", [2 * H * D], FP, kind="ExternalInput")
    Wb = nc.dram_tensor("Wb", [BD, H], F8, kind="ExternalInput")
    bb = nc.dram_tensor("bb", [H], FP, kind="ExternalInput")
    Wp = nc.dram_tensor("Wp", [H * D, CQ], FP, kind="ExternalInput")
    bp = nc.dram_tensor("bp", [CQ], FP, kind="ExternalInput")
    y = nc.dram_tensor("y", [NQC, CQ], FP, kind="ExternalOutput")

    with TileContext(nc) as tc:
        with (
            tc.tile_pool(name="const", bufs=1) as const,
            tc.tile_pool(name="zpool", bufs=12) as zpool,
            tc.tile_pool(name="xpool", bufs=3) as xpool,
            tc.tile_pool(name="epool", bufs=3) as epool,
            tc.tile_pool(name="atpool", bufs=4) as atpool,
            tc.tile_pool(name="proj_ps", bufs=2, space="PSUM") as proj_ps,
            tc.tile_pool(name="b_ps", bufs=3, space="PSUM") as b_psp,
            tc.tile_pool(name="t_ps", bufs=2, space="PSUM") as t_psp,
            tc.tile_pool(name="o_ps", bufs=1, space="PSUM") as o_psp,
        ):
            wb_sb = const.tile([P, H], F8)
            nc.sync.dma_start(wb_sb, Wb[:])
            zpre = []
            for gidx in range(3):
                z_sb = zpool.tile([P, GK, NQC], F8, tag="z", name=f"zpre{gidx}")
                nc.sync.dma_start(z_sb, zT[gidx])
                zpre.append(z_sb)

            wq_sb = const.tile([P, 2, H * D], BF)
            nc.sync.dma_start(wq_sb, Wq.rearrange("(o p) m -> p o m", p=P))
            wkv_sb = const.tile([P, 2, 2 * H * D], BF)
            nc.sync.dma_start(wkv_sb, Wkv.rearrange("(o p) m -> p o m", p=P))
            wp_sb = const.tile([P, 2, CQ], FP)
            nc.sync.dma_start(wp_sb, Wp.rearrange("(o p) m -> p o m", p=P))
            xqT_sb = const.tile([P, 2, NQC], BF)
            nc.sync.dma_start(xqT_sb, xqT.rearrange("(o p) q -> p o q", p=P))
            xkvT_sb = const.tile([P, 2, nk], BF)
            nc.sync.dma_start(xkvT_sb, xkvT.rearrange("(o p) k -> p o k", p=P))
            bq_sb = const.tile([P, 2], FP)
            nc.sync.dma_start(bq_sb, bq.rearrange("(o p) -> p o", p=P))
            bkvK_sb = const.tile([P, 2], FP)
            nc.sync.dma_start(bkvK_sb, bkv[0:H * D].rearrange("(o p) -> p o", p=P))
            bkvV_sb = const.tile([1, H * D], FP)
            nc.sync.dma_start(bkvV_sb, bkv[None, H * D:2 * H * D])
            bp_sb = const.tile([1, CQ], FP)
            nc.sync.dma_start(bp_sb, bp[None, :])
            bb_ap = bb[:]
            bb_sb = const.tile([P, H], FP)
            nc.gpsimd.dma_start(
                out=bb_sb,
                in_=bass.AP(tensor=bb_ap.tensor, offset=bb_ap.offset,
                            ap=[[0, P]] + list(bb_ap.ap)),
            )
            ident = const.tile([P, P], FP)
            make_identity(nc, ident)
            ident_bf = const.tile([P, P], BF)
            make_identity(nc, ident_bf)
            ones_row = const.tile([1, P], FP)
            nc.vector.memset(ones_row, 1.0)

            # HAM warmup: dense dummy matmuls while the first DMAs are in
            # flight; depends only on a vector-engine memset (gpsimd
            # identity takes ~7us to start).
            warm_sb = const.tile([P, P], BF)
            nc.vector.memset(warm_sb, 0.5)
            warm_ps = proj_ps.tile([P, 512], FP, tag="proj", name="warm")
            for w in range(48):
                nc.tensor.matmul(warm_ps[:, :P], lhsT=warm_sb, rhs=warm_sb,
                                 start=(w == 0), stop=(w == 47))

            vaug_sb = const.tile([P, kc_n, H, D + 1], BF)
            nc.vector.memset(vaug_sb, 1.0)

            qT_sb = const.tile([P, 2, NQC], BF)
            for m in range(2):
                ps = proj_ps.tile([P, 512], FP, tag="proj")
                for c in range(2):
                    nc.tensor.matmul(ps[:, :NQC],
                                     lhsT=wq_sb[:, c, m * P:(m + 1) * P],
                                     rhs=xqT_sb[:, c, :],
                                     start=(c == 0), stop=(c == 1))
                nc.vector.tensor_scalar(qT_sb[:, m, :], ps[:, :NQC],
                                        bq_sb[:, m:m + 1], SCALE * BIAS_SCALE,
                                        add, mult)

            kT_sb = const.tile([P, 2, nk], BF)
            for m in range(2):
                for nh in range((nk + 511) // 512):
                    nn_ = min(512, nk - nh * 512)
                    ps = proj_ps.tile([P, 512], FP, tag="proj")
                    for c in range(2):
                        nc.tensor.matmul(ps[:, :nn_],
                                         lhsT=wkv_sb[:, c, m * P:(m + 1) * P],
                                         rhs=xkvT_sb[:, c, nh * 512:nh * 512 + nn_],
                                         start=(c == 0), stop=(c == 1))
                    nc.vector.tensor_scalar(kT_sb[:, m, nh * 512:nh * 512 + nn_],
                                            ps[:, :nn_], bkvK_sb[:, m:m + 1],
                                            None, add)

            s_sb = const.tile([P, H, nk], FP)

            def emit_s(h, nh):
                """One 512-key S slab for head h: QK matmul + bias copy."""
                r0 = (h % 4) * 32
                ps = proj_ps.tile([P, 512], FP, tag="proj", name="qk_ps")
                nc.tensor.matmul(ps[:, :512],
                                 lhsT=qT_sb[r0:r0 + 32, h // 4, :],
                                 rhs=kT_sb[r0:r0 + 32, h // 4,
                                           nh * 512:nh * 512 + 512],
                                 start=True, stop=True,
                                 tile_position=(r0, 0))
                if (h * 2 + nh) % 2 == 0:
                    nc.scalar.activation(
                        s_sb[:, h, nh * 512:nh * 512 + 512], ps[:, :512],
                        mybir.ActivationFunctionType.Identity,
                        bias=bb_sb[:, h:h + 1])
                else:
                    nc.vector.tensor_scalar(
                        s_sb[:, h, nh * 512:nh * 512 + 512], ps[:, :512],
                        bb_sb[:, h:h + 1], None, add)

            # keys 0-511 up front; keys 512-1023 are sprinkled into the
            # first loop iterations (needed from chunk 4 onward).
            for h in range(H):
                emit_s(h, 0)

            for kc in range(kc_n):
                ps = proj_ps.tile([P, 512], FP, tag="proj", name="v_ps")
                for c in range(2):
                    nc.tensor.matmul(ps[:, :H * D],
                                     lhsT=xkvT_sb[:, c, kc * P:(kc + 1) * P],
                                     rhs=wkv_sb[:, c, H * D:2 * H * D],
                                     start=(c == 0), stop=False)
                nc.tensor.matmul(ps[:, :H * D], lhsT=ones_row,
                                 rhs=bkvV_sb, start=False, stop=True)
                nc.scalar.activation(
                    vaug_sb[:, kc, :, 0:D],
                    ps[:, :H * D].rearrange("p (h d) -> p h d", h=H),
                    mybir.ActivationFunctionType.Copy)

            o_ps = o_psp.tile([P, H * (D + 1)], FP)
            HKT = 64

            def emit_t(kc, x_sb):
                """Transposes + at-copies for chunk kc (x_sb is ready)."""
                ats = []
                for hg in range(2):
                    t_ps = t_psp.tile([P, 4, P], BF, tag="t")
                    for hl in range(4):
                        nc.tensor.transpose(t_ps[:, hl, :],
                                            x_sb[:, hg * 4 + hl, :], ident_bf)
                    at_sb = atpool.tile([P, 4, P], BF, tag="at")
                    nc.vector.tensor_copy(at_sb, t_ps)
                    ats.append(at_sb)
                return ats

            def emit_av(kc, ats):
                for hg in range(2):
                    for hl in range(4):
                        h = hg * 4 + hl
                        nc.tensor.matmul(
                            o_ps[:, h * (D + 1):(h + 1) * (D + 1)],
                            lhsT=ats[hg][:, hl, :], rhs=vaug_sb[:, kc, h, :],
                            start=(kc == 0 and h == 0),
                            stop=(kc == kc_n - 1 and h == H - 1))

            def emit_half(kc, hf, x_sb):
                b_ps = b_psp.tile([P, HKT * H], FP, tag="b")
                gidx = kc * 2 + hf
                if gidx < len(zpre):
                    z_sb = zpre[gidx]
                else:
                    z_sb = zpool.tile([P, gk, NQC], F8, tag="z")
                    nc.sync.dma_start(z_sb, zT[gidx])
                for t in range(HKT):
                    nc.tensor.matmul(b_ps[:, t * H:(t + 1) * H],
                                     lhsT=z_sb[:, t, :], rhs=wb_sb,
                                     start=(t == 0), stop=(t == HKT - 1))
                e_sb = epool.tile([P, H, HKT], FP, tag="e")
                nc.vector.tensor_tensor(
                    e_sb,
                    s_sb[:, :, kc * P + hf * HKT:kc * P + (hf + 1) * HKT],
                    b_ps.rearrange("p (kt h) -> p h kt", h=H), add)
                nc.scalar.activation(x_sb[:, :, hf * HKT:(hf + 1) * HKT],
                                     e_sb,
                                     mybir.ActivationFunctionType.Exp,
                                     scale=1.0 / BIAS_SCALE)

            # Steady-state PE order per chunk: zA(kc), T(kc-1), zB(kc),
            # AV(kc-1) — the add+exp for each half always completes under
            # the opposite half's z matmuls, so the PE never waits on it.
            prev = None
            for kc in range(kc_n):
                x_sb = xpool.tile([P, H, P], BF, tag="x")
                emit_half(kc, 0, x_sb)
                ats = emit_t(kc - 1, prev) if prev is not None else None
                # late S slabs (keys 512-1023) fill early-loop PE slack
                if kc < 2:
                    for h in range(4 * kc, 4 * kc + 4):
                        emit_s(h, 1)
                emit_half(kc, 1, x_sb)
                if ats is not None:
                    emit_av(kc - 1, ats)
                prev = x_sb
            ats = emit_t(kc_n - 1, prev)
            emit_av(kc_n - 1, ats)

            recip_sb = const.tile([P, H], FP)
            for h in range(H):
                nc.vector.reciprocal(recip_sb[:, h:h + 1],
                                     o_ps[:, h * (D + 1) + D:h * (D + 1) + D + 1])
            o_sb = const.tile([P, 2, P], FP)
            for h in range(H):
                nc.vector.tensor_scalar(
                    o_sb[:, h // 4, (h % 4) * 32:(h % 4) * 32 + 32],
                    o_ps[:, h * (D + 1):h * (D + 1) + D],
                    recip_sb[:, h:h + 1], None, mult)
            oT_sb = const.tile([P, 2, P], FP)
            for m in range(2):
                t_full = proj_ps.tile([P, 512], FP, tag="proj", name="t_full")
                t_ps = t_full[:, :P]
                nc.tensor.transpose(t_ps, o_sb[:, m, :], ident)
                nc.vector.tensor_copy(oT_sb[:, m, :], t_ps)
            ps = proj_ps.tile([P, 512], FP, tag="proj")
            for m in range(2):
                nc.tensor.matmul(ps[:, :CQ], lhsT=oT_sb[:, m, :],
                                 rhs=wp_sb[:, m, :], start=(m == 0), stop=False)
            nc.tensor.matmul(ps[:, :CQ], lhsT=ones_row, rhs=bp_sb,
                             start=False, stop=True)
            y_sb = const.tile([P, CQ], FP)
            nc.vector.tensor_copy(y_sb, ps[:, :CQ])
            nc.sync.dma_start(y[:], y_sb)

    nc.compile()
    return nc


def prep_inputs(x_q, x_kv, z, Wq, bq, Wkv, bkv, Wb, bb, Wp, bp,
                nk=1024, gk=GK):
    ng = nk // gk
    xkvT = np.ascontiguousarray(x_kv[0].T).astype(NP_BF)
    shared = dict(xkvT=xkvT,
                  Wq=np.ascontiguousarray(Wq).astype(NP_BF),
                  bq=np.ascontiguousarray(bq, dtype=np.float32),
                  Wkv=np.ascontiguousarray(Wkv).astype(NP_BF),
                  bkv=np.ascontiguousarray(bkv, dtype=np.float32),
                  Wb=(np.asarray(Wb, dtype=np.float32) * WB_SCALE
                      ).astype(NP_F8),
                  bb=np.asarray(bb, dtype=np.float32) * BIAS_SCALE,
                  Wp=np.ascontiguousarray(Wp, dtype=np.float32),
                  bp=np.ascontiguousarray(bp, dtype=np.float32))
    in_maps = []
    for i in range(NCORES):
        qs = i * NQC
        zi = z[0, qs:qs + NQC]
        zi = zi.reshape(NQC, ng, gk, BD).transpose(1, 3, 2, 0)
        in_maps.append(dict(
            zT=(np.ascontiguousarray(zi) * np.float32(Z_SCALE)
                ).astype(NP_F8),
            xqT=np.ascontiguousarray(x_q[0, qs:qs + NQC].T).astype(NP_BF),
            **shared,
        ))
    return in_maps


_NC_CACHE = {}


def kernel(x_q, x_kv, z, Wq, bq, Wkv, bkv, Wb, bb, Wp, bp):
    key = "full"
    if key not in _NC_CACHE:
        _NC_CACHE[key] = build_program()
    nc = _NC_CACHE[key]
    in_maps = prep_inputs(x_q, x_kv, z, Wq, bq, Wkv, bkv, Wb, bb, Wp, bp)
    res = run_bass_kernel_spmd(nc, in_maps, list(range(NCORES)))
    out = np.empty((1, NQ, CQ), dtype=np.float32)
    for i in range(NCORES):
        out[0, i * NQC:(i + 1) * NQC, :] = res.results[i]["y"]
    return out


# revision 48
# speedup vs baseline: 1.0067x; 1.0067x over previous
"""BiasAttention TRN2 kernel — q-sharded, fp8 z, baseline loop structure.

Known-good probe variant (the 114us run): fp8-e3m4 z, gk=64 DMA groups,
HAM warmup, original S-prologue + in-loop bias/add/exp/transpose/AV.
"""

import sys

if "/opt/trn_rl_repo" not in sys.path:
    sys.path.insert(0, "/opt/trn_rl_repo")

import ml_dtypes
import numpy as np

import concourse.bass as bass
import concourse.mybir as mybir
from concourse import bacc
from concourse.bass_utils import run_bass_kernel_spmd
from concourse.masks import make_identity
from concourse.tile import TileContext

P = 128
H = 8
D = 32
CQ = 256
CKV = 256
BD = 128
NQ = 1024
NCORES = 8
NQC = NQ // NCORES
SCALE = D ** (-0.5)

GK = 64
FP = mybir.dt.float32
BF = mybir.dt.bfloat16
F8 = mybir.dt.float8e3
NP_BF = ml_dtypes.bfloat16
NP_F8 = ml_dtypes.float8_e3m4

Z_SCALE = 2.0
WB_SCALE = 32.0
BIAS_SCALE = Z_SCALE * WB_SCALE


def build_program(nk=1024, gk=GK):
    kc_n = nk // P
    ng = nk // gk
    add = mybir.AluOpType.add
    mult = mybir.AluOpType.mult

    nc = bacc.Bacc("TRN2", target_bir_lowering=False, debug=False,
                   num_devices=NCORES)

    zT = nc.dram_tensor("zT", [ng, BD, gk, NQC], F8, kind="ExternalInput")
    xqT = nc.dram_tensor("xqT", [CQ, NQC], BF, kind="ExternalInput")
    xkvT = nc.dram_tensor("xkvT", [CKV, nk], BF, kind="ExternalInput")
    Wq = nc.dram_tensor("Wq", [CQ, H * D], BF, kind="ExternalInput")
    bq = nc.dram_tensor("bq", [H * D], FP, kind="ExternalInput")
    Wkv = nc.dram_tensor("Wkv", [CKV, 2 * H * D], BF, kind="ExternalInput")
    bkv = nc.dram_tensor("bkv", [2 * H * D], FP, kind="ExternalInput")
    Wb = nc.dram_tensor("Wb", [BD, H], F8, kind="ExternalInput")
    bb = nc.dram_tensor("bb", [H], FP, kind="ExternalInput")
    Wp = nc.dram_tensor("Wp", [H * D, CQ], FP, kind="ExternalInput")
    bp = nc.dram_tensor("bp", [CQ], FP, kind="ExternalInput")
    y = nc.dram_tensor("y", [NQC, CQ], FP, kind="ExternalOutput")

    with TileContext(nc) as tc:
        with (
            tc.tile_pool(name="const", bufs=1) as const,
            tc.tile_pool(name="zpool", bufs=12) as zpool,
            tc.tile_pool(name="xpool", bufs=3) as xpool,
            tc.tile_pool(name="epool", bufs=3) as epool,
            tc.tile_pool(name="atpool", bufs=4) as atpool,
            tc.tile_pool(name="proj_ps", bufs=2, space="PSUM") as proj_ps,
            tc.tile_pool(name="b_ps", bufs=3, space="PSUM") as b_psp,
            tc.tile_pool(name="t_ps", bufs=2, space="PSUM") as t_psp,
            tc.tile_pool(name="o_ps", bufs=1, space="PSUM") as o_psp,
        ):
            wb_sb = const.tile([P, H], F8)
            nc.sync.dma_start(wb_sb, Wb[:])
            # First 3 z groups lead the ring for a head start; weights are
            # queued before the rest of the z stream so a buffer-slot wait
            # on the sync queue can never starve them (12 upfront = pool
            # depth, so none of these waits; the last 4 issue in-loop).
            zlist = []

            def z_fetch(gidx):
                z_sb = zpool.tile([P, GK, NQC], F8, tag="z", name=f"zg{gidx}")
                nc.sync.dma_start(z_sb, zT[gidx])
                zlist.append(z_sb)

            for gidx in range(3):
                z_fetch(gidx)

            wq_sb = const.tile([P, 2, H * D], BF)
            nc.sync.dma_start(wq_sb, Wq.rearrange("(o p) m -> p o m", p=P))
            wkv_sb = const.tile([P, 2, 2 * H * D], BF)
            nc.sync.dma_start(wkv_sb, Wkv.rearrange("(o p) m -> p o m", p=P))
            wp_sb = const.tile([P, 2, CQ], FP)
            nc.sync.dma_start(wp_sb, Wp.rearrange("(o p) m -> p o m", p=P))
            xqT_sb = const.tile([P, 2, NQC], BF)
            nc.sync.dma_start(xqT_sb, xqT.rearrange("(o p) q -> p o q", p=P))
            xkvT_sb = const.tile([P, 2, nk], BF)
            nc.sync.dma_start(xkvT_sb, xkvT.rearrange("(o p) k -> p o k", p=P))
            bq_sb = const.tile([P, 2], FP)
            nc.sync.dma_start(bq_sb, bq.rearrange("(o p) -> p o", p=P))
            bkvK_sb = const.tile([P, 2], FP)
            nc.sync.dma_start(bkvK_sb, bkv[0:H * D].rearrange("(o p) -> p o", p=P))
            bkvV_sb = const.tile([1, H * D], FP)
            nc.sync.dma_start(bkvV_sb, bkv[None, H * D:2 * H * D])
            bp_sb = const.tile([1, CQ], FP)
            nc.sync.dma_start(bp_sb, bp[None, :])
            bb_ap = bb[:]
            bb_sb = const.tile([P, H], FP)
            nc.gpsimd.dma_start(
                out=bb_sb,
                in_=bass.AP(tensor=bb_ap.tensor, offset=bb_ap.offset,
                            ap=[[0, P]] + list(bb_ap.ap)),
            )
            ident = const.tile([P, P], FP)
            make_identity(nc, ident)
            ident_bf = const.tile([P, P], BF)
            make_identity(nc, ident_bf)
            ones_row = const.tile([1, P], FP)
            nc.vector.memset(ones_row, 1.0)
            # rest of the upfront z prefetch, behind the weight DMAs
            for gidx in range(3, 12):
                z_fetch(gidx)

            # HAM warmup: dense dummy matmuls while the first DMAs are in
            # flight; depends only on a vector-engine memset (gpsimd
            # identity takes ~7us to start).
            warm_sb = const.tile([P, P], BF)
            nc.vector.memset(warm_sb, 0.5)
            warm_ps = proj_ps.tile([P, 512], FP, tag="proj", name="warm")
            for w in range(48):
                nc.tensor.matmul(warm_ps[:, :P], lhsT=warm_sb, rhs=warm_sb,
                                 start=(w == 0), stop=(w == 47))

            vaug_sb = const.tile([P, kc_n, H, D + 1], BF)
            nc.vector.memset(vaug_sb, 1.0)

            qT_sb = const.tile([P, 2, NQC], BF)
            for m in range(2):
                ps = proj_ps.tile([P, 512], FP, tag="proj")
                for c in range(2):
                    nc.tensor.matmul(ps[:, :NQC],
                                     lhsT=wq_sb[:, c, m * P:(m + 1) * P],
                                     rhs=xqT_sb[:, c, :],
                                     start=(c == 0), stop=(c == 1))
                nc.vector.tensor_scalar(qT_sb[:, m, :], ps[:, :NQC],
                                        bq_sb[:, m:m + 1], SCALE * BIAS_SCALE,
                                        add, mult)

            kT_sb = const.tile([P, 2, nk], BF)
            for m in range(2):
                for nh in range((nk + 511) // 512):
                    nn_ = min(512, nk - nh * 512)
                    ps = proj_ps.tile([P, 512], FP, tag="proj")
                    for c in range(2):
                        nc.tensor.matmul(ps[:, :nn_],
                                         lhsT=wkv_sb[:, c, m * P:(m + 1) * P],
                                         rhs=xkvT_sb[:, c, nh * 512:nh * 512 + nn_],
                                         start=(c == 0), stop=(c == 1))
                    nc.vector.tensor_scalar(kT_sb[:, m, nh * 512:nh * 512 + nn_],
                                            ps[:, :nn_], bkvK_sb[:, m:m + 1],
                                            None, add)

            s_sb = const.tile([P, H, nk], FP)

            def emit_s(h, nh):
                """One 512-key S slab for head h: QK matmul + bias copy."""
                r0 = (h % 4) * 32
                ps = proj_ps.tile([P, 512], FP, tag="proj", name="qk_ps")
                nc.tensor.matmul(ps[:, :512],
                                 lhsT=qT_sb[r0:r0 + 32, h // 4, :],
                                 rhs=kT_sb[r0:r0 + 32, h // 4,
                                           nh * 512:nh * 512 + 512],
                                 start=True, stop=True,
                                 tile_position=(r0, 0))
                if (h * 2 + nh) % 2 == 0:
                    nc.scalar.activation(
                        s_sb[:, h, nh * 512:nh * 512 + 512], ps[:, :512],
                        mybir.ActivationFunctionType.Identity,
                        bias=bb_sb[:, h:h + 1])
                else:
                    nc.vector.tensor_scalar(
                        s_sb[:, h, nh * 512:nh * 512 + 512], ps[:, :512],
                        bb_sb[:, h:h + 1], None, add)

            # keys 0-511 up front; keys 512-1023 are sprinkled into the
            # first loop iterations (needed from chunk 4 onward).
            for h in range(H):
                emit_s(h, 0)

            for kc in range(kc_n):
                ps = proj_ps.tile([P, 512], FP, tag="proj", name="v_ps")
                for c in range(2):
                    nc.tensor.matmul(ps[:, :H * D],
                                     lhsT=xkvT_sb[:, c, kc * P:(kc + 1) * P],
                                     rhs=wkv_sb[:, c, H * D:2 * H * D],
                                     start=(c == 0), stop=False)
                nc.tensor.matmul(ps[:, :H * D], lhsT=ones_row,
                                 rhs=bkvV_sb, start=False, stop=True)
                nc.scalar.activation(
                    vaug_sb[:, kc, :, 0:D],
                    ps[:, :H * D].rearrange("p (h d) -> p h d", h=H),
                    mybir.ActivationFunctionType.Copy)

            o_ps = o_psp.tile([P, H * (D + 1)], FP)
            HKT = 64

            def emit_t(kc, x_sb):
                """Transposes + at-copies for chunk kc (x_sb is ready)."""
                ats = []
                for hg in range(2):
                    t_ps = t_psp.tile([P, 4, P], BF, tag="t")
                    for hl in range(4):
                        nc.tensor.transpose(t_ps[:, hl, :],
                                            x_sb[:, hg * 4 + hl, :], ident_bf)
                    at_sb = atpool.tile([P, 4, P], BF, tag="at")
                    nc.vector.tensor_copy(at_sb, t_ps)
                    ats.append(at_sb)
                return ats

            def emit_av(kc, ats):
                for hg in range(2):
                    for hl in range(4):
                        h = hg * 4 + hl
                        nc.tensor.matmul(
                            o_ps[:, h * (D + 1):(h + 1) * (D + 1)],
                            lhsT=ats[hg][:, hl, :], rhs=vaug_sb[:, kc, h, :],
                            start=(kc == 0 and h == 0),
                            stop=(kc == kc_n - 1 and h == H - 1))

            def emit_half(kc, hf, x_sb):
                b_ps = b_psp.tile([P, HKT * H], FP, tag="b")
                z_sb = zlist[kc * 2 + hf]
                for t in range(HKT):
                    nc.tensor.matmul(b_ps[:, t * H:(t + 1) * H],
                                     lhsT=z_sb[:, t, :], rhs=wb_sb,
                                     start=(t == 0), stop=(t == HKT - 1))
                e_sb = epool.tile([P, H, HKT], FP, tag="e")
                nc.vector.tensor_tensor(
                    e_sb,
                    s_sb[:, :, kc * P + hf * HKT:kc * P + (hf + 1) * HKT],
                    b_ps.rearrange("p (kt h) -> p h kt", h=H), add)
                nc.scalar.activation(x_sb[:, :, hf * HKT:(hf + 1) * HKT],
                                     e_sb,
                                     mybir.ActivationFunctionType.Exp,
                                     scale=1.0 / BIAS_SCALE)

            # Steady-state PE order per chunk: zA(kc), T(kc-1), zB(kc),
            # AV(kc-1) — each half's add+exp completes under the opposite
            # half's z matmuls, so the PE never waits on the exp chain.
            # The Tile scheduler's cost model mispredicts (no LDWEIGHTS
            # model, serial DMA), so the order is pinned with manual
            # virtual-time stamps (tile_wait_until in fake "ms" units).
            prev = None
            for kc in range(kc_n):
                base = 1.0 + kc
                x_sb = xpool.tile([P, H, P], BF, tag="x")
                with tc.tile_wait_until(base):
                    for gidx in (kc * 2 + 12, kc * 2 + 13):
                        if gidx < ng:
                            z_fetch(gidx)
                    emit_half(kc, 0, x_sb)
                ats = None
                if prev is not None:
                    with tc.tile_wait_until(base + 0.25):
                        ats = emit_t(kc - 1, prev)
                # late S slabs (keys 512-1023) fill early-loop PE slack
                if kc < 2:
                    with tc.tile_wait_until(base + 0.3):
                        for h in range(4 * kc, 4 * kc + 4):
                            emit_s(h, 1)
                with tc.tile_wait_until(base + 0.5):
                    emit_half(kc, 1, x_sb)
                if ats is not None:
                    with tc.tile_wait_until(base + 0.75):
                        emit_av(kc - 1, ats)
                prev = x_sb
            with tc.tile_wait_until(1.0 + kc_n):
                ats = emit_t(kc_n - 1, prev)
                emit_av(kc_n - 1, ats)

            with tc.tile_wait_until(2.0 + kc_n):
                recip_sb = const.tile([P, H], FP)
                for h in range(H):
                    nc.vector.reciprocal(
                        recip_sb[:, h:h + 1],
                        o_ps[:, h * (D + 1) + D:h * (D + 1) + D + 1])
                o_sb = const.tile([P, 2, P], FP)
                for h in range(H):
                    nc.vector.tensor_scalar(
                        o_sb[:, h // 4, (h % 4) * 32:(h % 4) * 32 + 32],
                        o_ps[:, h * (D + 1):h * (D + 1) + D],
                        recip_sb[:, h:h + 1], None, mult)
                oT_sb = const.tile([P, 2, P], FP)
                for m in range(2):
                    t_full = proj_ps.tile([P, 512], FP, tag="proj",
                                          name="t_full")
                    t_ps = t_full[:, :P]
                    nc.tensor.transpose(t_ps, o_sb[:, m, :], ident)
                    nc.vector.tensor_copy(oT_sb[:, m, :], t_ps)
                ps = proj_ps.tile([P, 512], FP, tag="proj")
                for m in range(2):
                    nc.tensor.matmul(ps[:, :CQ], lhsT=oT_sb[:, m, :],
                                     rhs=wp_sb[:, m, :], start=(m == 0),
                                     stop=False)
                nc.tensor.matmul(ps[:, :CQ], lhsT=ones_row, rhs=bp_sb,
                                 start=False, stop=True)
                y_sb = const.tile([P, CQ], FP)
                nc.vector.tensor_copy(y_sb, ps[:, :CQ])
                nc.sync.dma_start(y[:], y_sb)

    nc.compile()
    return nc


def prep_inputs(x_q, x_kv, z, Wq, bq, Wkv, bkv, Wb, bb, Wp, bp,
                nk=1024, gk=GK):
    ng = nk // gk
    xkvT = np.ascontiguousarray(x_kv[0].T).astype(NP_BF)
    shared = dict(xkvT=xkvT,
                  Wq=np.ascontiguousarray(Wq).astype(NP_BF),
                  bq=np.ascontiguousarray(bq, dtype=np.float32),
                  Wkv=np.ascontiguousarray(Wkv).astype(NP_BF),
                  bkv=np.ascontiguousarray(bkv, dtype=np.float32),
                  Wb=(np.asarray(Wb, dtype=np.float32) * WB_SCALE
                      ).astype(NP_F8),
                  bb=np.asarray(bb, dtype=np.float32) * BIAS_SCALE,
                  Wp=np.ascontiguousarray(Wp, dtype=np.float32),
                  bp=np.ascontiguousarray(bp, dtype=np.float32))
    in_maps = []
    for i in range(NCORES):
        qs = i * NQC
        zi = z[0, qs:qs + NQC]
        zi = zi.reshape(NQC, ng, gk, BD).transpose(1, 3, 2, 0)
        in_maps.append(dict(
            zT=(np.ascontiguousarray(zi) * np.float32(Z_SCALE)
                ).astype(NP_F8),
            xqT=np.ascontiguousarray(x_q[0, qs:qs + NQC].T).astype(NP_BF),
            **shared,
        ))
    return in_maps


_NC_CACHE = {}


def kernel(x_q, x_kv, z, Wq, bq, Wkv, bkv, Wb, bb, Wp, bp):
    key = "full"
    if key not in _NC_CACHE:
        _NC_CACHE[key] = build_program()
    nc = _NC_CACHE[key]
    in_maps = prep_inputs(x_q, x_kv, z, Wq, bq, Wkv, bkv, Wb, bb, Wp, bp)
    res = run_bass_kernel_spmd(nc, in_maps, list(range(NCORES)))
    out = np.empty((1, NQ, CQ), dtype=np.float32)
    for i in range(NCORES):
        out[0, i * NQC:(i + 1) * NQC, :] = res.results[i]["y"]
    return out
